# revision 28
# baseline (speedup 1.0000x reference)
"""Causal multi-head attention (B=2, T=2048, C=1024, H=16, d=64) on 8 trn2 cores.

Sharding: core i -> (batch b = i//4, head group g = i%4, 4 heads/core).
Data parallel over B, tensor parallel over heads; the out-proj partial sums
(contraction over this core's 256 channels) are reduced on the host during
the gather step, along with b_proj and the analytically-folded V bias.

Device kernel works entirely in [feature, token] (transposed) layout so no
on-device transposes are needed.

Perf design is driven by the PE HAM clock gate (2.4 GHz only under sustained
busy; recurring idle re-throttles to 1.2 GHz): the whole kernel is ONE
software pipeline in which projection work and the out-proj are fed to the
PE as filler inside the attention units, so the PE never starves while ACT
(exp, the second-busiest engine) chases it:

  QK(pair, tc4):  Q^T,K^T 512-token block for one head pair, k-loop paced
      by the bf16 x/w DMA stream at the front of the kernel.
  V(tc4):         4 V t-tiles (natural layout, stage-4 lhsT, both pairs'
      channels at once), ones column appended (row 64 = softmax Z).
  unit(pair, qb): attention g-cycles of 2 j-tiles x 2 heads: S^T (the two
      heads' K=64 matmuls land on the two 64-row PE tiles and run
      CONCURRENTLY when adjacent), causal mask (DVE), one exp per
      head-group on ACT (sliced so stale-psum cols are never consumed),
      PV accumulation lagged 2 g-cycles, then att^T = outT[0:64] * (1/Z)
      -> bf16 via reciprocal_approx_fast + GPSIMD partition_broadcast.
  Units run in ASCENDING qb order so unit (pair, qb) only needs q/k/v
      blocks 0..qb -- this is what lets projection/attention interleave.
  S6(qb): y^T = Wp.T @ att^T (bf16) as late filler; host sums partials.

x / W_qkv / W_v / W_proj travel as bf16 (halves DMA; ~1e-3 rel err), scores
and P stay fp32(r) end-to-end, accumulation always fp32 in PSUM.
"""

import numpy as np
import ml_dtypes

import concourse.bass as bass
import concourse.mybir as mybir
from concourse import bacc
import concourse.tile as tile
from concourse.bass_utils import run_bass_kernel_spmd

B, T, C, H, D = 2, 2048, 1024, 16, 64
NCORES = 8
HPC = 4            # heads per core
CS = HPC * D       # 256 channels per core (per Q/K/V block)
KT = C // 128      # 8 contraction tiles for the projections
NT = T // 128      # 16 token tiles of 128
QB = 512           # query block (psum bank width in fp32)
NQB = T // QB      # 4 query blocks
NEG = -1e9

F32 = mybir.dt.float32
F32R = mybir.dt.float32r
BF16 = mybir.dt.bfloat16

TRACE = False
LAST_RESULT = None


def _build_body(nc, tc, ctx, xT, wqk, wv, bqk, wp, masks, yT):
    AF = mybir.ActivationFunctionType

    persist = ctx.enter_context(tc.tile_pool(name="persist", bufs=1))

    xT_sb = persist.tile([128, KT, T], BF16, tag="xT", name="xT_sb")
    wqk_sb = persist.tile([128, KT, 2 * CS], BF16, tag="wqk", name="wqk_sb")
    wv_sb = persist.tile([128, KT, CS], BF16, tag="wv", name="wv_sb")
    bqk_sb = persist.tile([128, 4], F32, tag="bqk", name="bqk_sb")
    wp_sb = persist.tile([128, 2, C], BF16, tag="wp", name="wp_sb")
    mask_sb = persist.tile([128, 128], F32, tag="mask", name="mask_sb")
    qT_sb = [persist.tile([128, T], F32R, tag=f"qT{i}", name=f"qT{i}") for i in range(2)]
    kT_sb = [persist.tile([128, T], F32R, tag=f"kT{i}", name=f"kT{i}") for i in range(2)]
    v_sb = [persist.tile([128, HPC, D + 1], F32R, tag=f"v{t}", name=f"v{t}") for t in range(NT)]
    attT_sb = [persist.tile([128, T], BF16, tag=f"attT{i}", name=f"attT{i}") for i in range(2)]

    # PSUM: sT 2x2 banks + oT 2 + fill (shared stage1/stage6) 2 = 8 banks
    sTp = ctx.enter_context(tc.tile_pool(name="sT", bufs=2, space="PSUM"))
    oTp = ctx.enter_context(tc.tile_pool(name="outT", bufs=2, space="PSUM"))
    fillp = ctx.enter_context(tc.tile_pool(name="fillp", bufs=2, space="PSUM"))
    pTp = ctx.enter_context(tc.tile_pool(name="pT", bufs=10))
    smallp = ctx.enter_context(tc.tile_pool(name="small", bufs=8))
    ysp = ctx.enter_context(tc.tile_pool(name="ystage", bufs=6))

    # DMA order = consumption order; x lands in token-quarter slices so the
    # first QK/V chunks (and with them the whole pipeline) start after ~2MB
    # instead of waiting for the full input.
    def dma_x(tc4):
        nc.sync.dma_start(
            out=xT_sb[:, :, tc4 * QB:(tc4 + 1) * QB],
            in_=xT[:, :, tc4 * QB:(tc4 + 1) * QB],
        )
    # the startup transfers pace the whole pipeline: issue them from four
    # different engines so they ride four DMA queues in parallel instead of
    # serializing on the sync queue.
    engs = [nc.sync, nc.scalar, nc.gpsimd]
    for i, k0 in enumerate((0, 2, 4, 6)):
        engs[i % 3].dma_start(out=wqk_sb[:, k0:k0 + 2, :], in_=wqk[:, k0:k0 + 2, :])
        engs[(i + 1) % 3].dma_start(
            out=xT_sb[:, k0:k0 + 2, 0:QB], in_=xT[:, k0:k0 + 2, 0:QB])
    nc.gpsimd.dma_start(out=wv_sb[:, :, :], in_=wv[:, :, :])
    nc.sync.dma_start(out=bqk_sb[:, :], in_=bqk[:, :])
    nc.scalar.dma_start(
        out=xT_sb[:, :, QB:2 * QB], in_=xT[:, :, QB:2 * QB])
    nc.sync.dma_start(out=wp_sb[:, :, :], in_=wp[:, :, :])
    nc.sync.dma_start(out=mask_sb[:, :], in_=masks[:, :])
    dma_x(2)
    dma_x(3)

    ones_f32 = persist.tile([128, 4], F32, tag="ones_f32", name="ones_f32")
    nc.vector.memset(ones_f32[:, :], 1.0)
    for t in range(NT):
        nc.vector.tensor_copy(v_sb[t][:, :, D], ones_f32[:, :])

    # ---------------- projection chunks (filler) ----------------
    def qk_chunks(pair, tc4):
        def qk_chunk(ct, dst):
            def run():
                ps = fillp.tile([128, QB], F32, tag="fp", name="fp")
                for k in range(KT):
                    nc.tensor.matmul(
                        ps[:, :],
                        lhsT=wqk_sb[:, k, ct * 128:(ct + 1) * 128],
                        rhs=xT_sb[:, k, tc4 * QB:(tc4 + 1) * QB],
                        start=(k == 0),
                        stop=(k == KT - 1),
                    )
                nc.vector.tensor_scalar_add(
                    dst[:, tc4 * QB:(tc4 + 1) * QB], ps[:, :], bqk_sb[:, ct:ct + 1]
                )
            return run
        return [qk_chunk(pair, qT_sb[pair]), qk_chunk(2 + pair, kT_sb[pair])]

    def v_chunks(tc4):
        def v_chunk(t):
            def run():
                ps = fillp.tile([128, QB], F32, tag="fp", name="fp")
                for k in range(KT):
                    nc.tensor.matmul(
                        ps[:, 0:CS],
                        lhsT=xT_sb[:, k, t * 128:(t + 1) * 128],
                        rhs=wv_sb[:, k, :],
                        start=(k == 0),
                        stop=(k == KT - 1),
                    )
                nc.any.tensor_copy(
                    v_sb[t][:, :, 0:D],
                    ps[:, 0:CS].rearrange("p (h d) -> p h d", h=HPC),
                )
            return run
        return [v_chunk(t) for t in range(tc4 * 4, tc4 * 4 + 4)]

    # ---------------- stage 6 chunks (filler) ----------------
    def s6_chunks(sqb):
        def et_chunk(e2):
            def run():
                ys = ysp.tile([128, 2, QB], BF16, tag="ys", name="ys")
                for sub in range(2):
                    et = e2 * 2 + sub
                    yps_t = fillp.tile([128, QB], F32, tag="fp", name="fp")
                    for kc in range(2):
                        nc.tensor.matmul(
                            yps_t[:, :],
                            lhsT=wp_sb[:, kc, et * 128:(et + 1) * 128],
                            rhs=attT_sb[kc][:, sqb * QB:(sqb + 1) * QB],
                            start=(kc == 0),
                            stop=(kc == 1),
                        )
                    if e2 % 2 == 0:
                        nc.vector.tensor_copy(ys[:, sub, :], yps_t[:, :])
                    else:
                        nc.scalar.activation(ys[:, sub, :], yps_t[:, :], AF.Copy)
                nc.sync.dma_start(
                    out=yT[:, e2 * 2:e2 * 2 + 2, sqb * QB:(sqb + 1) * QB],
                    in_=ys[:, :, :],
                )
            return run
        return [et_chunk(e2) for e2 in range(C // 256)]

    # ---------------- attention unit ----------------
    def unit(pair, qb, filler):
        ktile, qtile = kT_sb[pair], qT_sb[pair]
        ngr = 2 * (qb + 1)
        njt = 4 * (qb + 1)
        oT = [oTp.tile([128, QB], F32, tag="oT", name="oT") for _ in range(2)]

        def emit_omms(m2, grp, pT):
            h = pair * 2 + m2
            for m in range(2):
                jt = grp * 2 + m
                c0 = 128 * (jt - 4 * qb) if grp >= ngr - 2 else 0
                nc.tensor.matmul(
                    oT[m2][0:D + 1, c0:QB],
                    lhsT=v_sb[jt][:, h, :],
                    rhs=pT[:, m, c0:QB],
                    start=(jt == 0),
                    stop=(jt == njt - 1),
                )

        pend = []
        for grp in range(ngr):
            diag = grp >= ngr - 2
            for m2 in range(2):
                po = m2 * D
                # the two heads' S-matmuls contract 64 partitions each on
                # disjoint ranges -> they run on the two 64-row PE tiles
                # concurrently when adjacent in the queue.
                sT = sTp.tile([128, 2, QB], F32, tag="sT", name="sT")
                for m in range(2):
                    jt = grp * 2 + m
                    c0 = 128 * (jt - 4 * qb) if diag else 0
                    c0s = min(c0, QB - 256)  # fp32r runs 4 cyc/row under
                    nc.tensor.matmul(      # 256 cols; overwidth is unread
                        sT[:, m, c0s:QB],
                        lhsT=ktile[po:po + D, jt * 128:(jt + 1) * 128],
                        rhs=qtile[po:po + D, qb * QB + c0s:(qb + 1) * QB],
                        start=True,
                        stop=True,
                    )
                if diag:  # causal mask on the two 128-wide diagonal blocks
                    for m in range(2):
                        c0 = 128 * (grp * 2 + m - 4 * qb)
                        nc.vector.tensor_add(
                            sT[:, m, c0:c0 + 128], sT[:, m, c0:c0 + 128], mask_sb[:, :]
                        )
                pT = pTp.tile([128, 2, QB], F32R, tag="pT", name="pT")
                # one exp per head-group; on diag groups slice from m0's
                # first valid column -- the m1 columns [c0, c0+128) this
                # covers are stale psum that no PV matmul ever reads.
                ce = 128 * (grp * 2 - 4 * qb) if diag else 0
                nc.scalar.activation(pT[:, :, ce:QB], sT[:, :, ce:QB], AF.Exp)
                pend.append((m2, grp, pT))
            if grp % 2 == 1:       # PV lags >=2 g-cycles behind exp and is
                # emitted every other cycle so 8 PV matmuls run back-to-back
                # in 128-row mode (halves the 64/128 tile-mode switch drains)
                while len(pend) > 4:
                    emit_omms(*pend.pop(0))
            if filler:             # spread filler over remaining g-cycles
                n = -(-len(filler) // (ngr - grp))
                for _ in range(n):
                    filler.pop(0)()
        while filler:
            filler.pop(0)()
        # flush + normalize per head so head 0's normalize chain overlaps
        # head 1's PV flush: att^T = outT[0:D] * (1/Z), Z = outT[D]
        for m2 in range(2):
            for ent in [e for e in pend if e[0] == m2]:
                emit_omms(*ent)
            po = m2 * D
            zrow = smallp.tile([1, QB], F32, tag="zrow", name="zrow")
            nc.vector.tensor_copy(zrow[:, :], oT[m2][D:D + 1, :])
            rz = smallp.tile([1, QB], F32, tag="rz", name="rz")
            nc.vector.reciprocal_approx_fast(out=rz[:, :], in_=zrow[:, :])
            zs = smallp.tile([D, QB], F32, tag="zs", name="zs")
            nc.gpsimd.partition_broadcast(zs[:, :], rz[:, :], channels=D)
            nc.vector.tensor_mul(
                attT_sb[pair][po:po + D, qb * QB:(qb + 1) * QB],
                oT[m2][0:D, :],
                zs[:, :],
            )

    # ---------------- the pipeline ----------------
    # fillers are scheduled as late as dependencies allow, to keep the PE
    # fed (warm) during the ACT-heavy later units.
    for c in qk_chunks(0, 0) + v_chunks(0):
        c()
    unit(0, 0, qk_chunks(1, 0))
    unit(1, 0, qk_chunks(0, 1))
    unit(0, 1, qk_chunks(1, 1) + v_chunks(1))
    unit(1, 1, qk_chunks(0, 2))
    unit(0, 2, qk_chunks(1, 2) + v_chunks(2))
    unit(1, 2, qk_chunks(0, 3) + v_chunks(3))
    unit(0, 3, qk_chunks(1, 3) + s6_chunks(1))
    unit(1, 3, s6_chunks(0) + s6_chunks(2))
    for c in s6_chunks(3):
        c()


def build_nc():
    from contextlib import ExitStack

    nc = bacc.Bacc("TRN2", target_bir_lowering=False)
    xT = nc.dram_tensor("xT", [128, KT, T], BF16, kind="ExternalInput")
    wqk = nc.dram_tensor("wqk", [128, KT, 2 * CS], BF16, kind="ExternalInput")
    wv = nc.dram_tensor("wv", [128, KT, CS], BF16, kind="ExternalInput")
    bqk = nc.dram_tensor("bqk", [128, 4], F32, kind="ExternalInput")
    wp = nc.dram_tensor("wp", [128, 2, C], BF16, kind="ExternalInput")
    masks = nc.dram_tensor("masks", [128, 128], F32, kind="ExternalInput")
    yT = nc.dram_tensor("yT", [128, C // 128, T], BF16, kind="ExternalOutput")
    with tile.TileContext(nc) as tc:
        with nc.allow_low_precision(reason="bf16 inputs / fp32r matmul; accumulation stays fp32 in PSUM"):
            with ExitStack() as ctx:
                _build_body(nc, tc, ctx, xT, wqk, wv, bqk, wp, masks, yT)
    nc.compile()
    return nc


def make_masks():
    r = np.arange(128)[:, None]
    c = np.arange(128)[None, :]
    return np.where(r <= c, np.float32(0.0), np.float32(NEG)).astype(np.float32)


def make_in_maps(x, W_qkv, b_qkv, W_proj):
    scale = np.float32(1.0 / np.sqrt(D))
    mask_h = make_masks()
    bf = ml_dtypes.bfloat16
    in_maps = []
    for i in range(NCORES):
        b, g = divmod(i, HPC)
        cs0 = g * CS
        wq = W_qkv[:, cs0:cs0 + CS] * scale
        wk = W_qkv[:, C + cs0:C + cs0 + CS]
        bq = b_qkv[cs0:cs0 + CS] * scale
        bk = b_qkv[C + cs0:C + cs0 + CS]
        def ktiles(a):  # [K*128, N] -> [128, K, N]
            return np.ascontiguousarray(
                a.reshape(-1, 128, a.shape[1]).transpose(1, 0, 2)
            )
        in_maps.append({
            "xT": ktiles(x[b].T).astype(bf),
            "wqk": ktiles(np.concatenate([wq, wk], axis=1)).astype(bf),
            "wv": ktiles(np.ascontiguousarray(W_qkv[:, 2 * C + cs0:2 * C + cs0 + CS])).astype(bf),
            "bqk": np.ascontiguousarray(
                np.concatenate([bq, bk]).reshape(4, 128).T
            ).astype(np.float32),
            "wp": ktiles(np.ascontiguousarray(W_proj[cs0:cs0 + CS, :])).astype(bf),
            "masks": mask_h,
        })
    return in_maps


_NC_CACHE = None


def _get_nc():
    global _NC_CACHE
    if _NC_CACHE is None:
        _NC_CACHE = build_nc()
    return _NC_CACHE


def gather(results, b_qkv, W_proj, b_proj):
    Y = np.zeros((B, T, C), np.float32)
    for i in range(NCORES):
        Y[i // HPC] += results[i]["yT"].transpose(1, 0, 2).reshape(C, T).T.astype(np.float32)
    Y += (b_qkv[2 * C:].astype(np.float32) @ W_proj.astype(np.float32)
          + b_proj.astype(np.float32))[None, None, :]
    return Y


def kernel(x, W_qkv, b_qkv, W_proj, b_proj):
    global LAST_RESULT
    x = np.asarray(x, np.float32)
    W_qkv = np.asarray(W_qkv, np.float32)
    b_qkv = np.asarray(b_qkv, np.float32)
    W_proj = np.asarray(W_proj, np.float32)
    b_proj = np.asarray(b_proj, np.float32)

    nc = _get_nc()
    in_maps = make_in_maps(x, W_qkv, b_qkv, W_proj)
    res = run_bass_kernel_spmd(nc, in_maps, list(range(NCORES)), trace=TRACE)
    LAST_RESULT = res
    if TRACE and res.exec_time_ns is not None:
        print(f"HW exec time: {res.exec_time_ns} ns")
    return gather(res.results, b_qkv, W_proj, b_proj)


# revision 29
# speedup vs baseline: 1.0336x; 1.0336x over previous
"""Causal multi-head attention (B=2, T=2048, C=1024, H=16, d=64) on 8 trn2 cores.

Sharding: core i -> (batch b = i//4, head group g = i%4, 4 heads/core).
Data parallel over B, tensor parallel over heads; the out-proj partial sums
(contraction over this core's 256 channels) are reduced on the host during
the gather step, along with b_proj and the analytically-folded V bias.

Device kernel works entirely in [feature, token] (transposed) layout so no
on-device transposes are needed.

Perf design is driven by the PE HAM clock gate (2.4 GHz only under sustained
busy; recurring idle re-throttles to 1.2 GHz): the whole kernel is ONE
software pipeline in which projection work and the out-proj are fed to the
PE as filler inside the attention units, so the PE never starves while ACT
(exp, the second-busiest engine) chases it:

  QK(pair, tc4):  Q^T,K^T 512-token block for one head pair, k-loop paced
      by the bf16 x/w DMA stream at the front of the kernel.
  V(tc4):         4 V t-tiles (natural layout, stage-4 lhsT, both pairs'
      channels at once), ones column appended (row 64 = softmax Z).
  unit(pair, qb): attention g-cycles of 2 j-tiles x 2 heads: S^T (the two
      heads' K=64 matmuls land on the two 64-row PE tiles and run
      CONCURRENTLY when adjacent), causal mask (DVE), one exp per
      head-group on ACT (sliced so stale-psum cols are never consumed),
      PV accumulation lagged 2 g-cycles, then att^T = outT[0:64] * (1/Z)
      -> bf16 via reciprocal_approx_fast + GPSIMD partition_broadcast.
  Units run in ASCENDING qb order so unit (pair, qb) only needs q/k/v
      blocks 0..qb -- this is what lets projection/attention interleave.
  S6(qb): y^T = Wp.T @ att^T (bf16) as late filler; host sums partials.

x / W_qkv / W_v / W_proj travel as bf16 (halves DMA; ~1e-3 rel err), scores
and P stay fp32(r) end-to-end, accumulation always fp32 in PSUM.
"""

import numpy as np
import ml_dtypes

import concourse.bass as bass
import concourse.mybir as mybir
from concourse import bacc
import concourse.tile as tile
from concourse.bass_utils import run_bass_kernel_spmd

B, T, C, H, D = 2, 2048, 1024, 16, 64
NCORES = 8
HPC = 4            # heads per core
CS = HPC * D       # 256 channels per core (per Q/K/V block)
KT = C // 128      # 8 contraction tiles for the projections
NT = T // 128      # 16 token tiles of 128
QB = 512           # query block (psum bank width in fp32)
NQB = T // QB      # 4 query blocks
NEG = -1e9

F32 = mybir.dt.float32
F32R = mybir.dt.float32r
BF16 = mybir.dt.bfloat16

TRACE = False
LAST_RESULT = None


def _build_body(nc, tc, ctx, xT, wqk, wv, bqk, wp, masks, yT):
    AF = mybir.ActivationFunctionType

    persist = ctx.enter_context(tc.tile_pool(name="persist", bufs=1))

    xT_sb = persist.tile([128, KT, T], BF16, tag="xT", name="xT_sb")
    wqk_sb = persist.tile([128, KT, 2 * CS], BF16, tag="wqk", name="wqk_sb")
    wv_sb = persist.tile([128, KT, CS], BF16, tag="wv", name="wv_sb")
    bqk_sb = persist.tile([128, 4], F32, tag="bqk", name="bqk_sb")
    wp_sb = persist.tile([128, 2, C], BF16, tag="wp", name="wp_sb")
    mask_sb = persist.tile([128, 128], F32, tag="mask", name="mask_sb")
    qT_sb = [persist.tile([128, T], F32R, tag=f"qT{i}", name=f"qT{i}") for i in range(2)]
    kT_sb = [persist.tile([128, T], F32R, tag=f"kT{i}", name=f"kT{i}") for i in range(2)]
    v_sb = [persist.tile([128, HPC, D + 1], F32R, tag=f"v{t}", name=f"v{t}") for t in range(NT)]
    attT_sb = [persist.tile([128, T], BF16, tag=f"attT{i}", name=f"attT{i}") for i in range(2)]

    # PSUM: sT 2x2 banks + oT 2 + fill (shared stage1/stage6) 2 = 8 banks
    sTp = ctx.enter_context(tc.tile_pool(name="sT", bufs=2, space="PSUM"))
    oTp = ctx.enter_context(tc.tile_pool(name="outT", bufs=2, space="PSUM"))
    fillp = ctx.enter_context(tc.tile_pool(name="fillp", bufs=2, space="PSUM"))
    pTp = ctx.enter_context(tc.tile_pool(name="pT", bufs=10))
    smallp = ctx.enter_context(tc.tile_pool(name="small", bufs=8))
    ysp = ctx.enter_context(tc.tile_pool(name="ystage", bufs=6))

    # DMA order = consumption order; x lands in token-quarter slices so the
    # first QK/V chunks (and with them the whole pipeline) start after ~2MB
    # instead of waiting for the full input.
    def dma_x(tc4):
        nc.sync.dma_start(
            out=xT_sb[:, :, tc4 * QB:(tc4 + 1) * QB],
            in_=xT[:, :, tc4 * QB:(tc4 + 1) * QB],
        )
    for k0 in (0, 2, 4, 6):
        nc.sync.dma_start(out=wqk_sb[:, k0:k0 + 2, :], in_=wqk[:, k0:k0 + 2, :])
        nc.sync.dma_start(
            out=xT_sb[:, k0:k0 + 2, 0:QB], in_=xT[:, k0:k0 + 2, 0:QB])
    nc.sync.dma_start(out=wv_sb[:, :, :], in_=wv[:, :, :])
    nc.sync.dma_start(out=bqk_sb[:, :], in_=bqk[:, :])
    dma_x(1)
    nc.sync.dma_start(out=wp_sb[:, :, :], in_=wp[:, :, :])
    nc.sync.dma_start(out=mask_sb[:, :], in_=masks[:, :])
    dma_x(2)
    dma_x(3)

    ones_f32 = persist.tile([128, 4], F32, tag="ones_f32", name="ones_f32")
    nc.vector.memset(ones_f32[:, :], 1.0)
    for t in range(NT):
        nc.vector.tensor_copy(v_sb[t][:, :, D], ones_f32[:, :])

    # ---------------- projection chunks (filler) ----------------
    def qk_chunks(pair, tc4):
        def qk_chunk(ct, dst):
            def run():
                ps = fillp.tile([128, QB], F32, tag="fp", name="fp")
                for k in range(KT):
                    nc.tensor.matmul(
                        ps[:, :],
                        lhsT=wqk_sb[:, k, ct * 128:(ct + 1) * 128],
                        rhs=xT_sb[:, k, tc4 * QB:(tc4 + 1) * QB],
                        start=(k == 0),
                        stop=(k == KT - 1),
                    )
                nc.vector.tensor_scalar_add(
                    dst[:, tc4 * QB:(tc4 + 1) * QB], ps[:, :], bqk_sb[:, ct:ct + 1]
                )
            return run
        return [qk_chunk(pair, qT_sb[pair]), qk_chunk(2 + pair, kT_sb[pair])]

    def v_chunks(tc4):
        def v_chunk(t):
            def run():
                ps = fillp.tile([128, QB], F32, tag="fp", name="fp")
                for k in range(KT):
                    nc.tensor.matmul(
                        ps[:, 0:CS],
                        lhsT=xT_sb[:, k, t * 128:(t + 1) * 128],
                        rhs=wv_sb[:, k, :],
                        start=(k == 0),
                        stop=(k == KT - 1),
                    )
                nc.any.tensor_copy(
                    v_sb[t][:, :, 0:D],
                    ps[:, 0:CS].rearrange("p (h d) -> p h d", h=HPC),
                )
            return run
        return [v_chunk(t) for t in range(tc4 * 4, tc4 * 4 + 4)]

    # ---------------- stage 6 chunks (filler) ----------------
    def s6_chunks(sqb):
        def et_chunk(e2):
            def run():
                ys = ysp.tile([128, 2, QB], BF16, tag="ys", name="ys")
                for sub in range(2):
                    et = e2 * 2 + sub
                    yps_t = fillp.tile([128, QB], F32, tag="fp", name="fp")
                    for kc in range(2):
                        nc.tensor.matmul(
                            yps_t[:, :],
                            lhsT=wp_sb[:, kc, et * 128:(et + 1) * 128],
                            rhs=attT_sb[kc][:, sqb * QB:(sqb + 1) * QB],
                            start=(kc == 0),
                            stop=(kc == 1),
                        )
                    if e2 % 2 == 0:
                        nc.vector.tensor_copy(ys[:, sub, :], yps_t[:, :])
                    else:
                        nc.scalar.activation(ys[:, sub, :], yps_t[:, :], AF.Copy)
                nc.sync.dma_start(
                    out=yT[:, e2 * 2:e2 * 2 + 2, sqb * QB:(sqb + 1) * QB],
                    in_=ys[:, :, :],
                )
            return run
        return [et_chunk(e2) for e2 in range(C // 256)]

    # ---------------- attention unit ----------------
    def unit(pair, qb, filler):
        ktile, qtile = kT_sb[pair], qT_sb[pair]
        ngr = 2 * (qb + 1)
        njt = 4 * (qb + 1)
        oT = [oTp.tile([128, QB], F32, tag="oT", name="oT") for _ in range(2)]

        def emit_omms(m2, grp, pT):
            h = pair * 2 + m2
            for m in range(2):
                jt = grp * 2 + m
                c0 = 128 * (jt - 4 * qb) if grp >= ngr - 2 else 0
                nc.tensor.matmul(
                    oT[m2][0:D + 1, c0:QB],
                    lhsT=v_sb[jt][:, h, :],
                    rhs=pT[:, m, c0:QB],
                    start=(jt == 0),
                    stop=(jt == njt - 1),
                )

        pend = []
        for grp in range(ngr):
            diag = grp >= ngr - 2
            for m2 in range(2):
                po = m2 * D
                # the two heads' S-matmuls contract 64 partitions each on
                # disjoint ranges -> they run on the two 64-row PE tiles
                # concurrently when adjacent in the queue.
                sT = sTp.tile([128, 2, QB], F32, tag="sT", name="sT")
                for m in range(2):
                    jt = grp * 2 + m
                    c0 = 128 * (jt - 4 * qb) if diag else 0
                    c0s = min(c0, QB - 256)  # fp32r runs 4 cyc/row under
                    nc.tensor.matmul(      # 256 cols; overwidth is unread
                        sT[:, m, c0s:QB],
                        lhsT=ktile[po:po + D, jt * 128:(jt + 1) * 128],
                        rhs=qtile[po:po + D, qb * QB + c0s:(qb + 1) * QB],
                        start=True,
                        stop=True,
                    )
                if diag:  # causal mask on the two 128-wide diagonal blocks
                    for m in range(2):
                        c0 = 128 * (grp * 2 + m - 4 * qb)
                        nc.vector.tensor_add(
                            sT[:, m, c0:c0 + 128], sT[:, m, c0:c0 + 128], mask_sb[:, :]
                        )
                pT = pTp.tile([128, 2, QB], F32R, tag="pT", name="pT")
                # one exp per head-group; on diag groups slice from m0's
                # first valid column -- the m1 columns [c0, c0+128) this
                # covers are stale psum that no PV matmul ever reads.
                ce = 128 * (grp * 2 - 4 * qb) if diag else 0
                nc.scalar.activation(pT[:, :, ce:QB], sT[:, :, ce:QB], AF.Exp)
                pend.append((m2, grp, pT))
            if grp % 2 == 1:       # PV lags >=2 g-cycles behind exp and is
                # emitted every other cycle so 8 PV matmuls run back-to-back
                # in 128-row mode (halves the 64/128 tile-mode switch drains)
                while len(pend) > 4:
                    emit_omms(*pend.pop(0))
            if filler:             # spread filler over remaining g-cycles
                n = -(-len(filler) // (ngr - grp))
                for _ in range(n):
                    filler.pop(0)()
        while filler:
            filler.pop(0)()
        # flush + normalize per head so head 0's normalize chain overlaps
        # head 1's PV flush: att^T = outT[0:D] * (1/Z), Z = outT[D]
        for m2 in range(2):
            for ent in [e for e in pend if e[0] == m2]:
                emit_omms(*ent)
            po = m2 * D
            zrow = smallp.tile([1, QB], F32, tag="zrow", name="zrow")
            nc.vector.tensor_copy(zrow[:, :], oT[m2][D:D + 1, :])
            rz = smallp.tile([1, QB], F32, tag="rz", name="rz")
            nc.vector.reciprocal_approx_fast(out=rz[:, :], in_=zrow[:, :])
            zs = smallp.tile([D, QB], F32, tag="zs", name="zs")
            nc.gpsimd.partition_broadcast(zs[:, :], rz[:, :], channels=D)
            nc.vector.tensor_mul(
                attT_sb[pair][po:po + D, qb * QB:(qb + 1) * QB],
                oT[m2][0:D, :],
                zs[:, :],
            )

    # ---------------- the pipeline ----------------
    # fillers are scheduled as late as dependencies allow, to keep the PE
    # fed (warm) during the ACT-heavy later units.
    for c in qk_chunks(0, 0) + v_chunks(0):
        c()
    unit(0, 0, qk_chunks(1, 0))
    unit(1, 0, qk_chunks(0, 1))
    unit(0, 1, qk_chunks(1, 1) + v_chunks(1))
    unit(1, 1, qk_chunks(0, 2))
    unit(0, 2, qk_chunks(1, 2) + v_chunks(2))
    unit(1, 2, qk_chunks(0, 3) + v_chunks(3))
    unit(0, 3, qk_chunks(1, 3) + s6_chunks(1))
    unit(1, 3, s6_chunks(0) + s6_chunks(2))
    for c in s6_chunks(3):
        c()


def build_nc():
    from contextlib import ExitStack

    nc = bacc.Bacc("TRN2", target_bir_lowering=False)
    xT = nc.dram_tensor("xT", [128, KT, T], BF16, kind="ExternalInput")
    wqk = nc.dram_tensor("wqk", [128, KT, 2 * CS], BF16, kind="ExternalInput")
    wv = nc.dram_tensor("wv", [128, KT, CS], BF16, kind="ExternalInput")
    bqk = nc.dram_tensor("bqk", [128, 4], F32, kind="ExternalInput")
    wp = nc.dram_tensor("wp", [128, 2, C], BF16, kind="ExternalInput")
    masks = nc.dram_tensor("masks", [128, 128], F32, kind="ExternalInput")
    yT = nc.dram_tensor("yT", [128, C // 128, T], BF16, kind="ExternalOutput")
    with tile.TileContext(nc) as tc:
        with nc.allow_low_precision(reason="bf16 inputs / fp32r matmul; accumulation stays fp32 in PSUM"):
            with ExitStack() as ctx:
                _build_body(nc, tc, ctx, xT, wqk, wv, bqk, wp, masks, yT)
    nc.compile()
    return nc


def make_masks():
    r = np.arange(128)[:, None]
    c = np.arange(128)[None, :]
    return np.where(r <= c, np.float32(0.0), np.float32(NEG)).astype(np.float32)


def make_in_maps(x, W_qkv, b_qkv, W_proj):
    scale = np.float32(1.0 / np.sqrt(D))
    mask_h = make_masks()
    bf = ml_dtypes.bfloat16
    in_maps = []
    for i in range(NCORES):
        b, g = divmod(i, HPC)
        cs0 = g * CS
        wq = W_qkv[:, cs0:cs0 + CS] * scale
        wk = W_qkv[:, C + cs0:C + cs0 + CS]
        bq = b_qkv[cs0:cs0 + CS] * scale
        bk = b_qkv[C + cs0:C + cs0 + CS]
        def ktiles(a):  # [K*128, N] -> [128, K, N]
            return np.ascontiguousarray(
                a.reshape(-1, 128, a.shape[1]).transpose(1, 0, 2)
            )
        in_maps.append({
            "xT": ktiles(x[b].T).astype(bf),
            "wqk": ktiles(np.concatenate([wq, wk], axis=1)).astype(bf),
            "wv": ktiles(np.ascontiguousarray(W_qkv[:, 2 * C + cs0:2 * C + cs0 + CS])).astype(bf),
            "bqk": np.ascontiguousarray(
                np.concatenate([bq, bk]).reshape(4, 128).T
            ).astype(np.float32),
            "wp": ktiles(np.ascontiguousarray(W_proj[cs0:cs0 + CS, :])).astype(bf),
            "masks": mask_h,
        })
    return in_maps


_NC_CACHE = None


def _get_nc():
    global _NC_CACHE
    if _NC_CACHE is None:
        _NC_CACHE = build_nc()
    return _NC_CACHE


def gather(results, b_qkv, W_proj, b_proj):
    Y = np.zeros((B, T, C), np.float32)
    for i in range(NCORES):
        Y[i // HPC] += results[i]["yT"].transpose(1, 0, 2).reshape(C, T).T.astype(np.float32)
    Y += (b_qkv[2 * C:].astype(np.float32) @ W_proj.astype(np.float32)
          + b_proj.astype(np.float32))[None, None, :]
    return Y


def kernel(x, W_qkv, b_qkv, W_proj, b_proj):
    global LAST_RESULT
    x = np.asarray(x, np.float32)
    W_qkv = np.asarray(W_qkv, np.float32)
    b_qkv = np.asarray(b_qkv, np.float32)
    W_proj = np.asarray(W_proj, np.float32)
    b_proj = np.asarray(b_proj, np.float32)

    nc = _get_nc()
    in_maps = make_in_maps(x, W_qkv, b_qkv, W_proj)
    res = run_bass_kernel_spmd(nc, in_maps, list(range(NCORES)), trace=TRACE)
    LAST_RESULT = res
    if TRACE and res.exec_time_ns is not None:
        print(f"HW exec time: {res.exec_time_ns} ns")
    return gather(res.results, b_qkv, W_proj, b_proj)
